# revision 5
# baseline (speedup 1.0000x reference)
"""Trainium2 Bass kernel for the Clifford (geometric) product on Cl(3,0).

out[n,k] = sum_{i,j} S[i,j,k] * a[n,i] * b[n,j]

Algorithm: Cl(3,0) ~= Mat2(C) (Pauli matrices). Per multivector:
  - basis elements indexed by GF(2)^3 masks; host permutes the 8 columns into
    a kernel-friendly order (and folds a global 1/2 scale into `a`)
  - 6 transform ops build the 2x2 complex matrix entries U/V (8 adds/mv each)
  - 8 product ops compute the 32 real products of the complex 2x2 matmul
  - a 3-level add/sub tree (structure signs folded into op choice) reduces to
    the 8 output components, written in a layout the host un-permutes
80 elementwise ops/mv total (vs 120 direct), every tree/transform op is
unit-stride inner (bf16 2x DVE mode); products are 1x (one broadcast operand)
and are split between the Vector and GpSimd engines by multivector range.

IO is bf16 (host rounds f32 inputs, upcasts the output); the rel-err budget
(2e-2) comfortably covers bf16 rounding. Host<->device runs through a cached
shard_map jit so repeat calls skip retracing, with the donated output buffer
created device-side (no zero upload).
"""

import os

os.environ.setdefault("BY_DEFAULT_DISABLE_SUBTILE_DEPS", "1")

import numpy as np
import ml_dtypes

import concourse.bass as bass
import concourse.bacc as bacc
import concourse.mybir as mybir
from concourse.tile import TileContext

# ---------------------------------------------------------------- geometry
N_TOTAL = 4194304
N_CORES = 8
NC = N_TOTAL // N_CORES        # 524288 multivectors per core
P = 128                        # partitions
E = 512                        # multivectors per partition per tile
EG = 352                       # of E, how many go to the GpSimd product lane
N_TILES = NC // (P * E)        # 8

F32 = mybir.dt.float32
BF16 = mybir.dt.bfloat16
ADD = mybir.AluOpType.add
SUB = mybir.AluOpType.subtract
MUL = mybir.AluOpType.mult

# host column gather/scatter (mask-space basis ordering; see module docstring)
IN_COLS = [0, 4, 1, 2, 3, 7, 5, 6]    # kernel input slot s <- ref column
OUT_COLS = [0, 1, 4, 2, 7, 6, 3, 5]   # kernel output pos p -> ref column

# ---- per-mv op tables (offsets/dims within one mv's 8-elem block) ----
# transforms: (alu, out_off, out_dims, in0_off, in0_dims, in1_off, in1_dims)
TRANS_A = [
    (ADD, 0, [(1, 2)], 0, [(1, 2)], 4, [(1, 2)]),
    (SUB, 2, [(1, 2)], 2, [(1, 2)], 6, [(1, 2)]),
    (ADD, 4, [(1, 2)], 2, [(1, 2)], 6, [(1, 2)]),
    (SUB, 6, [(1, 2)], 0, [(1, 2)], 4, [(1, 2)]),
]
TRANS_B = [
    (ADD, 0, [(1, 4)], 0, [(1, 4)], 4, [(1, 4)]),
    (SUB, 4, [(1, 4)], 2, [(-2, 2), (1, 2)], 6, [(-2, 2), (1, 2)]),
]
# products: per term t, V-pattern (voff, vstep); 2 half-ops each (parts 0-3/4-7)
PROD_T = [(0, 1), (1, -1), (2, 1), (3, -1)]
# tree L1 part-patterns
L1_NEG = [(4, 2), (3, 2)]   # parts {0,3,4,7}
L1_POS = [(4, 2), (1, 2)]   # parts {1,2,5,6} (offset +1)


def _mkap(base, dims, offset):
    ap = base.copy()
    part = list(base.ap[0])
    ap.ap = mybir.VecI64Pair([part] + [[d, c] for (d, c) in dims])
    ap.offset = base.offset + offset
    return ap


def build_nc(nc_mv=NC, e=E, eg=EG):
    n_tiles = nc_mv // (P * e)
    assert n_tiles * P * e == nc_mv
    ed = e - eg

    nc = bacc.Bacc("TRN2", target_bir_lowering=False, debug=False)
    a_d = nc.dram_tensor("a", [nc_mv, 8], BF16, kind="ExternalInput")
    b_d = nc.dram_tensor("b", [nc_mv, 8], BF16, kind="ExternalInput")
    o_d = nc.dram_tensor("o", [nc_mv, 8], BF16, kind="ExternalOutput")

    a_v = a_d.ap().rearrange("(t p e) c -> t p (e c)", t=n_tiles, p=P)
    b_v = b_d.ap().rearrange("(t p e) c -> t p (e c)", t=n_tiles, p=P)
    o_v = o_d.ap().rearrange("(t p e) c -> t p (e c)", t=n_tiles, p=P)

    with TileContext(nc) as tc:
        with (
            tc.tile_pool(name="io", bufs=2) as io_pool,
            tc.tile_pool(name="uv", bufs=2) as uv_pool,
            tc.tile_pool(name="pr", bufs=2) as pr_pool,
        ):
            for t in range(n_tiles):
                a_t = io_pool.tile([P, 8 * e], BF16, tag="a")
                b_t = io_pool.tile([P, 8 * e], BF16, tag="b")
                o_t = io_pool.tile([P, 8 * e], BF16, tag="o")
                u_t = uv_pool.tile([P, 8 * e], BF16, tag="u")
                v_t = uv_pool.tile([P, 8 * e], BF16, tag="v")

                nc.sync.dma_start(out=a_t[:, :], in_=a_v[t])
                nc.scalar.dma_start(out=b_t[:, :], in_=b_v[t])

                # ---- transforms (DVE, full tile) ----
                for (alu, oo, od, i0o, i0d, i1o, i1d) in TRANS_A:
                    nc.vector.tensor_tensor(
                        out=_mkap(u_t, [(8, e)] + od, oo),
                        in0=_mkap(a_t, [(8, e)] + i0d, i0o),
                        in1=_mkap(a_t, [(8, e)] + i1d, i1o), op=alu)
                for (alu, oo, od, i0o, i0d, i1o, i1d) in TRANS_B:
                    nc.vector.tensor_tensor(
                        out=_mkap(v_t, [(8, e)] + od, oo),
                        in0=_mkap(b_t, [(8, e)] + i0d, i0o),
                        in1=_mkap(b_t, [(8, e)] + i1d, i1o), op=alu)

                # ---- per-range products + tree ----
                ranges = []
                if ed > 0:
                    ranges.append(("d", nc.vector, 0, ed))
                if eg > 0:
                    ranges.append(("g", nc.gpsimd, ed, eg))

                ptiles = {}
                for rname, eng, m0, cnt in ranges:
                    pts = [pr_pool.tile([P, 8 * cnt], BF16, tag=f"p{q}{rname}",
                                        name=f"p{q}{rname}")
                           for q in range(4)]
                    ptiles[rname] = pts
                    base = m0 * 8
                    for q, (voff, vs) in enumerate(PROD_T):
                        for half in range(2):
                            eng.tensor_tensor(
                                out=_mkap(pts[q], [(8, cnt), (1, 4)], 4 * half),
                                in0=_mkap(u_t, [(8, cnt), (0, 4)],
                                          base + q + 4 * half),
                                in1=_mkap(v_t, [(8, cnt), (4, 2), (vs, 2)],
                                          base + voff),
                                op=MUL)

                for rname, eng, m0, cnt in ranges:
                    pts = ptiles[rname]
                    c_t = pr_pool.tile([P, 8 * cnt], BF16, tag=f"c{rname}",
                                       name=f"c{rname}")
                    mv = [(8, cnt)]
                    V = nc.vector
                    # L1 (in place into P0 / P2)
                    V.tensor_tensor(out=_mkap(pts[0], mv + L1_NEG, 0),
                                    in0=_mkap(pts[0], mv + L1_NEG, 0),
                                    in1=_mkap(pts[1], mv + L1_NEG, 0), op=SUB)
                    V.tensor_tensor(out=_mkap(pts[0], mv + L1_POS, 1),
                                    in0=_mkap(pts[0], mv + L1_POS, 1),
                                    in1=_mkap(pts[1], mv + L1_POS, 1), op=ADD)
                    V.tensor_tensor(out=_mkap(pts[2], mv + L1_NEG, 0),
                                    in0=_mkap(pts[2], mv + L1_NEG, 0),
                                    in1=_mkap(pts[3], mv + L1_NEG, 0), op=ADD)
                    V.tensor_tensor(out=_mkap(pts[2], mv + L1_POS, 1),
                                    in0=_mkap(pts[2], mv + L1_POS, 1),
                                    in1=_mkap(pts[3], mv + L1_POS, 1), op=SUB)
                    # L2
                    V.tensor_tensor(out=_mkap(c_t, mv + [(1, 8)], 0),
                                    in0=_mkap(pts[0], mv + [(1, 8)], 0),
                                    in1=_mkap(pts[2], mv + [(1, 8)], 0), op=ADD)
                    # final -> o_t (kernel position order)
                    base = m0 * 8
                    V.tensor_tensor(
                        out=_mkap(o_t, mv + [(1, 4)], base),
                        in0=_mkap(c_t, mv + [(1, 2), (4, 2)], 0),
                        in1=_mkap(c_t, mv + [(1, 2), (-4, 2)], 6), op=ADD)
                    V.tensor_tensor(
                        out=_mkap(o_t, mv + [(1, 4)], base + 4),
                        in0=_mkap(c_t, mv + [(-1, 2), (4, 2)], 1),
                        in1=_mkap(c_t, mv + [(-1, 2), (-4, 2)], 7), op=SUB)

                nc.sync.dma_start(out=o_v[t], in_=o_t[:, :])
    nc.compile()
    return nc


_NC_CACHE = {}


def _get_nc(nc_mv=NC, e=E, eg=EG):
    key = (nc_mv, e, eg)
    if key not in _NC_CACHE:
        _NC_CACHE[key] = build_nc(nc_mv, e, eg)
    return _NC_CACHE[key]


# ------------------------------------------------- cached PJRT execution
_EXEC_CACHE = {}


def _get_exec(nc, n_cores):
    """Cached equivalent of bass2jax.run_bass_via_pjrt: one traced+compiled
    shard_map jit per nc, with the donated output buffer made on device."""
    key = (id(nc), n_cores)
    if key in _EXEC_CACHE:
        return _EXEC_CACHE[key]

    import jax
    import jax.numpy as jnp
    from jax.experimental.shard_map import shard_map
    from jax.sharding import Mesh, PartitionSpec, NamedSharding
    from concourse import bass2jax

    bass2jax.install_neuronx_cc_hook()

    partition_name = (nc.partition_id_tensor.name
                      if nc.partition_id_tensor else None)
    in_names, out_names, out_avals = [], [], []
    for alloc in nc.m.functions[0].allocations:
        if not isinstance(alloc, mybir.MemoryLocationSet):
            continue
        name = alloc.memorylocations[0].name
        if alloc.kind == "ExternalInput":
            if name != partition_name:
                in_names.append(name)
        elif alloc.kind == "ExternalOutput":
            shape = tuple(alloc.tensor_shape)
            dtype = mybir.dt.np(alloc.dtype)
            out_names.append(name)
            out_avals.append(jax.core.ShapedArray(shape, dtype))
    n_params = len(in_names)
    n_outs = len(out_avals)
    all_names = in_names + out_names
    if partition_name is not None:
        all_names.append(partition_name)
    donate = tuple(range(n_params, n_params + n_outs))

    def _body(*args):
        operands = list(args)
        if partition_name is not None:
            operands.append(bass2jax.partition_id_tensor())
        outs = bass2jax._bass_exec_p.bind(
            *operands,
            out_avals=tuple(out_avals),
            in_names=tuple(all_names),
            out_names=tuple(out_names),
            lowering_input_output_aliases=(),
            sim_require_finite=True,
            sim_require_nnan=True,
            nc=nc,
        )
        return tuple(outs)

    devices = jax.devices()[:n_cores]
    mesh = Mesh(np.asarray(devices), ("core",))
    in_specs = (PartitionSpec("core"),) * (n_params + n_outs)
    out_specs = (PartitionSpec("core"),) * n_outs
    sharded = jax.jit(
        shard_map(_body, mesh=mesh, in_specs=in_specs, out_specs=out_specs,
                  check_rep=False),
        donate_argnums=donate, keep_unused=True)

    out_sharding = NamedSharding(mesh, PartitionSpec("core"))
    zero_fns = [
        jax.jit(
            (lambda av: (lambda: jnp.zeros((n_cores * av.shape[0],
                                            *av.shape[1:]), av.dtype)))(av),
            out_shardings=out_sharding)
        for av in out_avals
    ]
    entry = (sharded, zero_fns, in_names)
    _EXEC_CACHE[key] = entry
    return entry


def _run_device(nc, a_bf, b_bf, n_cores):
    sharded, zero_fns, in_names = _get_exec(nc, n_cores)
    by_name = {"a": a_bf, "b": b_bf}
    inputs = [by_name[nm] for nm in in_names]
    zeros = [zf() for zf in zero_fns]
    out_arrs = sharded(*inputs, *zeros)
    return out_arrs[0]


# ------------------------------------------------------------- host casts
_BF16 = ml_dtypes.bfloat16


def _to_bf16(x32):
    """f32 (contiguous) -> bf16 with round-to-nearest-even-ish rounding."""
    u = x32.view(np.uint32)
    r = ((u + np.uint32(0x7FFF) + ((u >> np.uint32(16)) & np.uint32(1)))
         >> np.uint32(16)).astype(np.uint16)
    return r.view(_BF16)


def kernel(a, b, M=None, **_):
    a = np.asarray(a, dtype=np.float32)
    b = np.asarray(b, dtype=np.float32)
    n = a.shape[0]
    assert n % N_CORES == 0
    nc_mv = n // N_CORES
    nc = _get_nc(nc_mv, E, EG)

    a_pre = np.ascontiguousarray(a[:, IN_COLS]) * np.float32(0.5)
    b_pre = np.ascontiguousarray(b[:, IN_COLS])
    a_bf = _to_bf16(a_pre)
    b_bf = _to_bf16(b_pre)

    o_dev = _run_device(nc, a_bf, b_bf, N_CORES)
    o_bf = np.asarray(o_dev)  # (n, 8) bf16, kernel position order

    o32 = (o_bf.view(np.uint16).astype(np.uint32) << np.uint32(16)).view(
        np.float32)
    out = np.empty((n, 8), np.float32)
    out[:, OUT_COLS] = o32
    return out


# revision 11
# speedup vs baseline: 1.1804x; 1.1804x over previous
"""Trainium2 Bass kernel for the Clifford (geometric) product on Cl(3,0).

out[n,k] = sum_{i,j} S[i,j,k] * a[n,i] * b[n,j]

Algorithm: Cl(3,0) ~= Mat2(C) (Pauli matrices). Per multivector:
  - basis elements indexed by GF(2)^3 masks; host permutes the 8 columns into
    a kernel-friendly order (and folds a global 1/2 scale into `a`)
  - 6 transform ops build the 2x2 complex matrix entries U/V (8 adds/mv each)
  - 8 product ops compute the 32 real products of the complex 2x2 matmul
  - a 3-level add/sub tree (structure signs folded into op choice) reduces to
    the 8 output components, written in a layout the host un-permutes
80 elementwise ops/mv total (vs 120 direct), every tree/transform op is
unit-stride inner (bf16 2x DVE mode); products are 1x (one broadcast operand)
and are split between the Vector and GpSimd engines by multivector range.

IO is bf16 (host rounds f32 inputs, upcasts the output); the rel-err budget
(2e-2) comfortably covers bf16 rounding. Host<->device runs through a cached
shard_map jit so repeat calls skip retracing, with the donated output buffer
created device-side (no zero upload).
"""

import os

os.environ.setdefault("BY_DEFAULT_DISABLE_SUBTILE_DEPS", "1")

import numpy as np
import ml_dtypes

import concourse.bass as bass
import concourse.bacc as bacc
import concourse.mybir as mybir
from concourse.tile import TileContext

# ---------------------------------------------------------------- geometry
N_TOTAL = 4194304
N_CORES = 8
NC = N_TOTAL // N_CORES        # 524288 multivectors per core
P = 128                        # partitions
E = 512                        # multivectors per partition per tile
EG = 352                       # of E, how many go to the GpSimd product lane
N_TILES = NC // (P * E)        # 8

F32 = mybir.dt.float32
BF16 = mybir.dt.bfloat16
ADD = mybir.AluOpType.add
SUB = mybir.AluOpType.subtract
MUL = mybir.AluOpType.mult

# host column gather/scatter (mask-space basis ordering; see module docstring)
IN_COLS = [0, 4, 1, 2, 3, 7, 5, 6]    # kernel input slot s <- ref column
OUT_COLS = [0, 4, 1, 2, 3, 7, 5, 6]   # kernel output pos p -> ref column

# ---- per-mv op tables (offsets/dims within one mv's 8-elem block) ----
# transforms: (alu, out_off, out_dims, in0_off, in0_dims, in1_off, in1_dims)
# U and V transforms share one structure (adds in slots 0-3, subs in 4-7)
TRANS = [
    (ADD, 0, [(1, 4)], 0, [(1, 4)], 4, [(1, 4)]),
    (SUB, 4, [(1, 4)], 2, [(-2, 2), (1, 2)], 6, [(-2, 2), (1, 2)]),
]
# products: U' (dup'd) slot pairs per term; V offsets (q//2)*2, t1/t3 use V2
USLOT = [(0, 2), (1, 3), (4, 6), (5, 7)]
# tree L2 part-patterns (sigma01 split)
L2_NEG = [(4, 2), (3, 2)]    # parts {0,3,4,7}
L2_POS = [(4, 2), (-1, 2)]   # parts (2,1,6,5) at offset +2 (4B-aligned base)


def _mkap(base, dims, offset):
    ap = base.copy()
    part = list(base.ap[0])
    ap.ap = mybir.VecI64Pair([part] + [[d, c] for (d, c) in dims])
    ap.offset = base.offset + offset
    return ap


# instruction name -> (label, elems); populated at build time for profiling
OP_NAMES = {}


def _reg(label, elems, inst):
    OP_NAMES[inst.ins.name] = (label, elems)
    return inst


def build_nc(nc_mv=NC, e=E, eg=EG):
    n_tiles = nc_mv // (P * e)
    assert n_tiles * P * e == nc_mv
    ed = e - eg

    nc = bacc.Bacc("TRN2", target_bir_lowering=False, debug=False)
    a_d = nc.dram_tensor("a", [nc_mv, 8], BF16, kind="ExternalInput")
    b_d = nc.dram_tensor("b", [nc_mv, 8], BF16, kind="ExternalInput")
    o_d = nc.dram_tensor("o", [nc_mv, 8], BF16, kind="ExternalOutput")

    a_v = a_d.ap().rearrange("(t p e) c -> t p (e c)", t=n_tiles, p=P)
    b_v = b_d.ap().rearrange("(t p e) c -> t p (e c)", t=n_tiles, p=P)
    o_v = o_d.ap().rearrange("(t p e) c -> t p (e c)", t=n_tiles, p=P)

    with TileContext(nc) as tc:
        with (
            tc.tile_pool(name="io", bufs=2) as io_pool,
            tc.tile_pool(name="uv", bufs=1) as uv_pool,
            tc.tile_pool(name="pr", bufs=2) as pr_pool,
        ):
            def emit_products(u2_t, v_t, v2_t, ranges, ptiles):
                for rname, eng, m0, cnt in ranges:
                    pts = ptiles[rname]
                    base = m0 * 8
                    for q in range(4):
                        vt = v_t if q % 2 == 0 else v2_t
                        voff = (q // 2) * 2
                        for half in range(2):
                            _reg(f"prod{q}h{half}{rname}", cnt * 4,
                                 eng.tensor_tensor(
                                out=_mkap(pts[q], [(8, cnt), (1, 4)], 4 * half),
                                in0=_mkap(u2_t, [(16, cnt), (0, 2), (1, 2)],
                                          m0 * 16 + 2 * USLOT[q][half]),
                                in1=_mkap(vt, [(8, cnt), (4, 2), (1, 2)],
                                          base + voff),
                                op=MUL))

            def emit_tree(tidx, ranges, ptiles, o_t):
                for rname, eng, m0, cnt in ranges:
                    pts = ptiles[rname]
                    c_t = pr_pool.tile([P, 8 * cnt], BF16, tag=f"c{rname}",
                                       name=f"c{rname}")
                    mv = [(8, cnt)]
                    V = nc.vector
                    _reg(f"L1a{rname}", cnt * 8,
                         V.tensor_tensor(out=_mkap(pts[0], mv + [(1, 8)], 0),
                                    in0=_mkap(pts[0], mv + [(1, 8)], 0),
                                    in1=_mkap(pts[2], mv + [(1, 8)], 0), op=ADD))
                    _reg(f"L1b{rname}", cnt * 8,
                         V.tensor_tensor(out=_mkap(pts[1], mv + [(1, 8)], 0),
                                    in0=_mkap(pts[1], mv + [(1, 8)], 0),
                                    in1=_mkap(pts[3], mv + [(1, 8)], 0), op=SUB))
                    _reg(f"L2a{rname}", cnt * 4,
                         V.tensor_tensor(out=_mkap(c_t, mv + L2_NEG, 0),
                                    in0=_mkap(pts[0], mv + L2_NEG, 0),
                                    in1=_mkap(pts[1], mv + L2_NEG, 0), op=SUB))
                    _reg(f"L2b{rname}", cnt * 4,
                         V.tensor_tensor(out=_mkap(c_t, mv + L2_POS, 2),
                                    in0=_mkap(pts[0], mv + L2_POS, 2),
                                    in1=_mkap(pts[1], mv + L2_POS, 2), op=ADD))
                    base = m0 * 8
                    _reg(f"finA{rname}", cnt * 4, V.tensor_tensor(
                        out=_mkap(o_t, mv + [(1, 4)], base),
                        in0=_mkap(c_t, mv + [(4, 2), (1, 2)], 0),
                        in1=_mkap(c_t, mv + [(-4, 2), (1, 2)], 6), op=ADD))
                    _reg(f"finB{rname}", cnt * 4, V.tensor_tensor(
                        out=_mkap(o_t, mv + [(1, 4)], base + 4),
                        in0=_mkap(c_t, mv + [(4, 2), (1, 2)], 0),
                        in1=_mkap(c_t, mv + [(-4, 2), (1, 2)], 6), op=SUB))
                nc.sync.dma_start(out=o_v[tidx], in_=o_t[:, :])

            Copy = mybir.ActivationFunctionType.Copy
            pending = None
            for t in range(n_tiles):
                a_t = io_pool.tile([P, 8 * e], BF16, tag="a")
                b_t = io_pool.tile([P, 8 * e], BF16, tag="b")
                o_t = io_pool.tile([P, 8 * e], BF16, tag="o")
                u_t = uv_pool.tile([P, 8 * e], BF16, tag="u")
                v_t = uv_pool.tile([P, 8 * e], BF16, tag="v")
                u2_t = uv_pool.tile([P, 16 * e], BF16, tag="u2")
                v2_t = uv_pool.tile([P, 8 * e], BF16, tag="v2")

                nc.sync.dma_start(out=a_t[:, :], in_=a_v[t])
                nc.scalar.dma_start(out=b_t[:, :], in_=b_v[t])

                # ---- transforms (DVE, full tile) ----
                for nm, src_t, dst_t in (("A", a_t, u_t), ("B", b_t, v_t)):
                    for ti, (alu, oo, od, i0o, i0d, i1o, i1d) in enumerate(TRANS):
                        _reg(f"trans{nm}{ti}", e * 4, nc.vector.tensor_tensor(
                            out=_mkap(dst_t, [(8, e)] + od, oo),
                            in0=_mkap(src_t, [(8, e)] + i0d, i0o),
                            in1=_mkap(src_t, [(8, e)] + i1d, i1o), op=alu))

                # ---- ACT copies (overlap with previous tile's tree) ----
                _reg("dupU", e * 16, nc.scalar.activation(
                    out=_mkap(u2_t, [(16, e), (2, 8), (1, 2)], 0),
                    in_=_mkap(u_t, [(8, e), (1, 8), (0, 2)], 0), func=Copy))
                _reg("swapV", e * 8, nc.scalar.activation(
                    out=_mkap(v2_t, [(8, e), (1, 8)], 0),
                    in_=_mkap(v_t, [(8, e), (2, 4), (-1, 2)], 1), func=Copy))

                # ---- previous tile's tree while ACT runs ----
                if pending is not None:
                    emit_tree(*pending)

                ranges = []
                ed = e - eg
                if ed > 0:
                    ranges.append(("d", nc.vector, 0, ed))
                if eg > 0:
                    ranges.append(("g", nc.gpsimd, ed, eg))
                ptiles = {}
                for rname, eng, m0, cnt in ranges:
                    ptiles[rname] = [
                        pr_pool.tile([P, 8 * cnt], BF16, tag=f"p{q}{rname}",
                                     name=f"p{q}{rname}")
                        for q in range(4)]
                emit_products(u2_t, v_t, v2_t, ranges, ptiles)
                pending = (t, ranges, ptiles, o_t)
            emit_tree(*pending)
    nc.compile()
    return nc


_NC_CACHE = {}


def _get_nc(nc_mv=NC, e=E, eg=EG):
    key = (nc_mv, e, eg)
    if key not in _NC_CACHE:
        _NC_CACHE[key] = build_nc(nc_mv, e, eg)
    return _NC_CACHE[key]


# ------------------------------------------------- cached PJRT execution
_EXEC_CACHE = {}


def _get_exec(nc, n_cores):
    """Cached equivalent of bass2jax.run_bass_via_pjrt: one traced+compiled
    shard_map jit per nc, with the donated output buffer made on device."""
    key = (id(nc), n_cores)
    if key in _EXEC_CACHE:
        return _EXEC_CACHE[key]

    import jax
    import jax.numpy as jnp
    from jax.experimental.shard_map import shard_map
    from jax.sharding import Mesh, PartitionSpec, NamedSharding
    from concourse import bass2jax

    bass2jax.install_neuronx_cc_hook()

    partition_name = (nc.partition_id_tensor.name
                      if nc.partition_id_tensor else None)
    in_names, out_names, out_avals = [], [], []
    for alloc in nc.m.functions[0].allocations:
        if not isinstance(alloc, mybir.MemoryLocationSet):
            continue
        name = alloc.memorylocations[0].name
        if alloc.kind == "ExternalInput":
            if name != partition_name:
                in_names.append(name)
        elif alloc.kind == "ExternalOutput":
            shape = tuple(alloc.tensor_shape)
            dtype = mybir.dt.np(alloc.dtype)
            out_names.append(name)
            out_avals.append(jax.core.ShapedArray(shape, dtype))
    n_params = len(in_names)
    n_outs = len(out_avals)
    all_names = in_names + out_names
    if partition_name is not None:
        all_names.append(partition_name)
    donate = tuple(range(n_params, n_params + n_outs))

    def _body(*args):
        operands = list(args)
        if partition_name is not None:
            operands.append(bass2jax.partition_id_tensor())
        outs = bass2jax._bass_exec_p.bind(
            *operands,
            out_avals=tuple(out_avals),
            in_names=tuple(all_names),
            out_names=tuple(out_names),
            lowering_input_output_aliases=(),
            sim_require_finite=True,
            sim_require_nnan=True,
            nc=nc,
        )
        return tuple(outs)

    devices = jax.devices()[:n_cores]
    mesh = Mesh(np.asarray(devices), ("core",))
    in_specs = (PartitionSpec("core"),) * (n_params + n_outs)
    out_specs = (PartitionSpec("core"),) * n_outs
    sharded = jax.jit(
        shard_map(_body, mesh=mesh, in_specs=in_specs, out_specs=out_specs,
                  check_rep=False),
        donate_argnums=donate, keep_unused=True)

    out_sharding = NamedSharding(mesh, PartitionSpec("core"))
    zero_fns = [
        jax.jit(
            (lambda av: (lambda: jnp.zeros((n_cores * av.shape[0],
                                            *av.shape[1:]), av.dtype)))(av),
            out_shardings=out_sharding)
        for av in out_avals
    ]
    entry = (sharded, zero_fns, in_names)
    _EXEC_CACHE[key] = entry
    return entry


def _run_device(nc, a_bf, b_bf, n_cores):
    sharded, zero_fns, in_names = _get_exec(nc, n_cores)
    by_name = {"a": a_bf, "b": b_bf}
    inputs = [by_name[nm] for nm in in_names]
    zeros = [zf() for zf in zero_fns]
    out_arrs = sharded(*inputs, *zeros)
    return out_arrs[0]


# ------------------------------------------------------------- host casts
_BF16 = ml_dtypes.bfloat16


def _to_bf16(x32):
    """f32 (contiguous) -> bf16 with round-to-nearest-even-ish rounding."""
    u = x32.view(np.uint32)
    r = ((u + np.uint32(0x7FFF) + ((u >> np.uint32(16)) & np.uint32(1)))
         >> np.uint32(16)).astype(np.uint16)
    return r.view(_BF16)


def kernel(a, b, M=None, **_):
    a = np.asarray(a, dtype=np.float32)
    b = np.asarray(b, dtype=np.float32)
    n = a.shape[0]
    assert n % N_CORES == 0
    nc_mv = n // N_CORES
    nc = _get_nc(nc_mv, E, EG)

    a_pre = np.ascontiguousarray(a[:, IN_COLS]) * np.float32(0.5)
    b_pre = np.ascontiguousarray(b[:, IN_COLS])
    a_bf = _to_bf16(a_pre)
    b_bf = _to_bf16(b_pre)

    o_dev = _run_device(nc, a_bf, b_bf, N_CORES)
    o_bf = np.asarray(o_dev)  # (n, 8) bf16, kernel position order

    o32 = (o_bf.view(np.uint16).astype(np.uint32) << np.uint32(16)).view(
        np.float32)
    out = np.empty((n, 8), np.float32)
    out[:, OUT_COLS] = o32
    return out


# revision 12
# speedup vs baseline: 1.8894x; 1.6006x over previous
"""Trainium2 Bass kernel for the Clifford (geometric) product on Cl(3,0).

out[n,k] = sum_{i,j} S[i,j,k] * a[n,i] * b[n,j]

Algorithm: Cl(3,0) ~= Mat2(C) (Pauli matrices). Per multivector:
  - basis elements indexed by GF(2)^3 masks; host permutes the 8 columns into
    a kernel-friendly order (and folds a global 1/2 scale into `a`)
  - 6 transform ops build the 2x2 complex matrix entries U/V (8 adds/mv each)
  - 8 product ops compute the 32 real products of the complex 2x2 matmul
  - a 3-level add/sub tree (structure signs folded into op choice) reduces to
    the 8 output components, written in a layout the host un-permutes
80 elementwise ops/mv total (vs 120 direct), every tree/transform op is
unit-stride inner (bf16 2x DVE mode); products are 1x (one broadcast operand)
and are split between the Vector and GpSimd engines by multivector range.

IO is bf16 (host rounds f32 inputs, upcasts the output); the rel-err budget
(2e-2) comfortably covers bf16 rounding. Host<->device runs through a cached
shard_map jit so repeat calls skip retracing, with the donated output buffer
created device-side (no zero upload).
"""

import os

os.environ.setdefault("BY_DEFAULT_DISABLE_SUBTILE_DEPS", "1")

import numpy as np
import ml_dtypes

import concourse.bass as bass
import concourse.bacc as bacc
import concourse.mybir as mybir
from concourse.tile import TileContext

# ---------------------------------------------------------------- geometry
N_TOTAL = 4194304
N_CORES = 8
NC = N_TOTAL // N_CORES        # 524288 multivectors per core
P = 128                        # partitions
E = 512                        # multivectors per partition per tile
EG = 0                         # mvs per tile on the GpSimd lane (0: DVE only —
                               # concurrent GpSimd SBUF traffic slows DVE more
                               # than the offload saves)
N_TILES = NC // (P * E)        # 8

F32 = mybir.dt.float32
BF16 = mybir.dt.bfloat16
ADD = mybir.AluOpType.add
SUB = mybir.AluOpType.subtract
MUL = mybir.AluOpType.mult

# host column gather/scatter (mask-space basis ordering; see module docstring)
IN_COLS = [0, 4, 1, 2, 3, 7, 5, 6]    # kernel input slot s <- ref column
OUT_COLS = [0, 4, 1, 2, 3, 7, 5, 6]   # kernel output pos p -> ref column

# ---- per-mv op tables (offsets/dims within one mv's 8-elem block) ----
# transforms: (alu, out_off, out_dims, in0_off, in0_dims, in1_off, in1_dims)
# U and V transforms share one structure (adds in slots 0-3, subs in 4-7)
TRANS = [
    (ADD, 0, [(1, 4)], 0, [(1, 4)], 4, [(1, 4)]),
    (SUB, 4, [(1, 4)], 2, [(-2, 2), (1, 2)], 6, [(-2, 2), (1, 2)]),
]
# products: U' (dup'd) slot pairs per term; V offsets (q//2)*2, t1/t3 use V2
USLOT = [(0, 2), (1, 3), (4, 6), (5, 7)]
# tree L2 part-patterns (sigma01 split)
L2_NEG = [(4, 2), (3, 2)]    # parts {0,3,4,7}
L2_POS = [(4, 2), (-1, 2)]   # parts (2,1,6,5) at offset +2 (4B-aligned base)


def _mkap(base, dims, offset):
    ap = base.copy()
    part = list(base.ap[0])
    ap.ap = mybir.VecI64Pair([part] + [[d, c] for (d, c) in dims])
    ap.offset = base.offset + offset
    return ap


# instruction name -> (label, elems); populated at build time for profiling
OP_NAMES = {}


def _reg(label, elems, inst):
    OP_NAMES[inst.ins.name] = (label, elems)
    return inst


def build_nc(nc_mv=NC, e=E, eg=EG):
    n_tiles = nc_mv // (P * e)
    assert n_tiles * P * e == nc_mv
    ed = e - eg

    nc = bacc.Bacc("TRN2", target_bir_lowering=False, debug=False)
    a_d = nc.dram_tensor("a", [nc_mv, 8], BF16, kind="ExternalInput")
    b_d = nc.dram_tensor("b", [nc_mv, 8], BF16, kind="ExternalInput")
    o_d = nc.dram_tensor("o", [nc_mv, 8], BF16, kind="ExternalOutput")

    a_v = a_d.ap().rearrange("(t p e) c -> t p (e c)", t=n_tiles, p=P)
    b_v = b_d.ap().rearrange("(t p e) c -> t p (e c)", t=n_tiles, p=P)
    o_v = o_d.ap().rearrange("(t p e) c -> t p (e c)", t=n_tiles, p=P)

    with TileContext(nc) as tc:
        with (
            tc.tile_pool(name="io", bufs=2) as io_pool,
            tc.tile_pool(name="uv", bufs=1) as uv_pool,
            tc.tile_pool(name="pr", bufs=2) as pr_pool,
        ):
            def emit_products(u2_t, v_t, v2_t, ranges, ptiles):
                for rname, eng, m0, cnt in ranges:
                    pts = ptiles[rname]
                    base = m0 * 8
                    for q in range(4):
                        vt = v_t if q % 2 == 0 else v2_t
                        voff = (q // 2) * 2
                        for half in range(2):
                            _reg(f"prod{q}h{half}{rname}", cnt * 4,
                                 eng.tensor_tensor(
                                out=_mkap(pts[q], [(8, cnt), (1, 4)], 4 * half),
                                in0=_mkap(u2_t, [(16, cnt), (0, 2), (1, 2)],
                                          m0 * 16 + 2 * USLOT[q][half]),
                                in1=_mkap(vt, [(8, cnt), (4, 2), (1, 2)],
                                          base + voff),
                                op=MUL))

            def emit_tree(tidx, ranges, ptiles, o_t):
                for rname, eng, m0, cnt in ranges:
                    pts = ptiles[rname]
                    c_t = pr_pool.tile([P, 8 * cnt], BF16, tag=f"c{rname}",
                                       name=f"c{rname}")
                    mv = [(8, cnt)]
                    V = nc.vector
                    _reg(f"L1a{rname}", cnt * 8,
                         V.tensor_tensor(out=_mkap(pts[0], mv + [(1, 8)], 0),
                                    in0=_mkap(pts[0], mv + [(1, 8)], 0),
                                    in1=_mkap(pts[2], mv + [(1, 8)], 0), op=ADD))
                    _reg(f"L1b{rname}", cnt * 8,
                         V.tensor_tensor(out=_mkap(pts[1], mv + [(1, 8)], 0),
                                    in0=_mkap(pts[1], mv + [(1, 8)], 0),
                                    in1=_mkap(pts[3], mv + [(1, 8)], 0), op=SUB))
                    _reg(f"L2a{rname}", cnt * 4,
                         V.tensor_tensor(out=_mkap(c_t, mv + L2_NEG, 0),
                                    in0=_mkap(pts[0], mv + L2_NEG, 0),
                                    in1=_mkap(pts[1], mv + L2_NEG, 0), op=SUB))
                    _reg(f"L2b{rname}", cnt * 4,
                         V.tensor_tensor(out=_mkap(c_t, mv + L2_POS, 2),
                                    in0=_mkap(pts[0], mv + L2_POS, 2),
                                    in1=_mkap(pts[1], mv + L2_POS, 2), op=ADD))
                    base = m0 * 8
                    _reg(f"finA{rname}", cnt * 4, V.tensor_tensor(
                        out=_mkap(o_t, mv + [(1, 4)], base),
                        in0=_mkap(c_t, mv + [(4, 2), (1, 2)], 0),
                        in1=_mkap(c_t, mv + [(-4, 2), (1, 2)], 6), op=ADD))
                    _reg(f"finB{rname}", cnt * 4, V.tensor_tensor(
                        out=_mkap(o_t, mv + [(1, 4)], base + 4),
                        in0=_mkap(c_t, mv + [(4, 2), (1, 2)], 0),
                        in1=_mkap(c_t, mv + [(-4, 2), (1, 2)], 6), op=SUB))
                nc.sync.dma_start(out=o_v[tidx], in_=o_t[:, :])

            Copy = mybir.ActivationFunctionType.Copy
            pending = None
            for t in range(n_tiles):
                a_t = io_pool.tile([P, 8 * e], BF16, tag="a")
                b_t = io_pool.tile([P, 8 * e], BF16, tag="b")
                o_t = io_pool.tile([P, 8 * e], BF16, tag="o")
                u_t = uv_pool.tile([P, 8 * e], BF16, tag="u")
                v_t = uv_pool.tile([P, 8 * e], BF16, tag="v")
                u2_t = uv_pool.tile([P, 16 * e], BF16, tag="u2")
                v2_t = uv_pool.tile([P, 8 * e], BF16, tag="v2")

                nc.sync.dma_start(out=a_t[:, :], in_=a_v[t])
                nc.scalar.dma_start(out=b_t[:, :], in_=b_v[t])

                # ---- transforms (DVE, full tile) ----
                for nm, src_t, dst_t in (("A", a_t, u_t), ("B", b_t, v_t)):
                    for ti, (alu, oo, od, i0o, i0d, i1o, i1d) in enumerate(TRANS):
                        _reg(f"trans{nm}{ti}", e * 4, nc.vector.tensor_tensor(
                            out=_mkap(dst_t, [(8, e)] + od, oo),
                            in0=_mkap(src_t, [(8, e)] + i0d, i0o),
                            in1=_mkap(src_t, [(8, e)] + i1d, i1o), op=alu))

                # ---- ACT copies (overlap with previous tile's tree) ----
                _reg("dupU", e * 16, nc.scalar.activation(
                    out=_mkap(u2_t, [(16, e), (2, 8), (1, 2)], 0),
                    in_=_mkap(u_t, [(8, e), (1, 8), (0, 2)], 0), func=Copy))
                _reg("swapV", e * 8, nc.scalar.activation(
                    out=_mkap(v2_t, [(8, e), (1, 8)], 0),
                    in_=_mkap(v_t, [(8, e), (2, 4), (-1, 2)], 1), func=Copy))

                # ---- previous tile's tree while ACT runs ----
                if pending is not None:
                    emit_tree(*pending)

                ranges = []
                ed = e - eg
                if ed > 0:
                    ranges.append(("d", nc.vector, 0, ed))
                if eg > 0:
                    ranges.append(("g", nc.gpsimd, ed, eg))
                ptiles = {}
                for rname, eng, m0, cnt in ranges:
                    ptiles[rname] = [
                        pr_pool.tile([P, 8 * cnt], BF16, tag=f"p{q}{rname}",
                                     name=f"p{q}{rname}")
                        for q in range(4)]
                emit_products(u2_t, v_t, v2_t, ranges, ptiles)
                pending = (t, ranges, ptiles, o_t)
            emit_tree(*pending)
    nc.compile()
    return nc


_NC_CACHE = {}


def _get_nc(nc_mv=NC, e=E, eg=EG):
    key = (nc_mv, e, eg)
    if key not in _NC_CACHE:
        _NC_CACHE[key] = build_nc(nc_mv, e, eg)
    return _NC_CACHE[key]


# ------------------------------------------------- cached PJRT execution
_EXEC_CACHE = {}


def _get_exec(nc, n_cores):
    """Cached equivalent of bass2jax.run_bass_via_pjrt: one traced+compiled
    shard_map jit per nc, with the donated output buffer made on device."""
    key = (id(nc), n_cores)
    if key in _EXEC_CACHE:
        return _EXEC_CACHE[key]

    import jax
    import jax.numpy as jnp
    from jax.experimental.shard_map import shard_map
    from jax.sharding import Mesh, PartitionSpec, NamedSharding
    from concourse import bass2jax

    bass2jax.install_neuronx_cc_hook()

    partition_name = (nc.partition_id_tensor.name
                      if nc.partition_id_tensor else None)
    in_names, out_names, out_avals = [], [], []
    for alloc in nc.m.functions[0].allocations:
        if not isinstance(alloc, mybir.MemoryLocationSet):
            continue
        name = alloc.memorylocations[0].name
        if alloc.kind == "ExternalInput":
            if name != partition_name:
                in_names.append(name)
        elif alloc.kind == "ExternalOutput":
            shape = tuple(alloc.tensor_shape)
            dtype = mybir.dt.np(alloc.dtype)
            out_names.append(name)
            out_avals.append(jax.core.ShapedArray(shape, dtype))
    n_params = len(in_names)
    n_outs = len(out_avals)
    all_names = in_names + out_names
    if partition_name is not None:
        all_names.append(partition_name)
    donate = tuple(range(n_params, n_params + n_outs))

    def _body(*args):
        operands = list(args)
        if partition_name is not None:
            operands.append(bass2jax.partition_id_tensor())
        outs = bass2jax._bass_exec_p.bind(
            *operands,
            out_avals=tuple(out_avals),
            in_names=tuple(all_names),
            out_names=tuple(out_names),
            lowering_input_output_aliases=(),
            sim_require_finite=True,
            sim_require_nnan=True,
            nc=nc,
        )
        return tuple(outs)

    devices = jax.devices()[:n_cores]
    mesh = Mesh(np.asarray(devices), ("core",))
    in_specs = (PartitionSpec("core"),) * (n_params + n_outs)
    out_specs = (PartitionSpec("core"),) * n_outs
    sharded = jax.jit(
        shard_map(_body, mesh=mesh, in_specs=in_specs, out_specs=out_specs,
                  check_rep=False),
        donate_argnums=donate, keep_unused=True)

    out_sharding = NamedSharding(mesh, PartitionSpec("core"))
    zero_fns = [
        jax.jit(
            (lambda av: (lambda: jnp.zeros((n_cores * av.shape[0],
                                            *av.shape[1:]), av.dtype)))(av),
            out_shardings=out_sharding)
        for av in out_avals
    ]
    entry = (sharded, zero_fns, in_names)
    _EXEC_CACHE[key] = entry
    return entry


def _run_device(nc, a_bf, b_bf, n_cores):
    sharded, zero_fns, in_names = _get_exec(nc, n_cores)
    by_name = {"a": a_bf, "b": b_bf}
    inputs = [by_name[nm] for nm in in_names]
    zeros = [zf() for zf in zero_fns]
    out_arrs = sharded(*inputs, *zeros)
    return out_arrs[0]


# ------------------------------------------------------------- host casts
_BF16 = ml_dtypes.bfloat16


def _to_bf16(x32):
    """f32 (contiguous) -> bf16 with round-to-nearest-even-ish rounding."""
    u = x32.view(np.uint32)
    r = ((u + np.uint32(0x7FFF) + ((u >> np.uint32(16)) & np.uint32(1)))
         >> np.uint32(16)).astype(np.uint16)
    return r.view(_BF16)


def kernel(a, b, M=None, **_):
    a = np.asarray(a, dtype=np.float32)
    b = np.asarray(b, dtype=np.float32)
    n = a.shape[0]
    assert n % N_CORES == 0
    nc_mv = n // N_CORES
    nc = _get_nc(nc_mv, E, EG)

    a_pre = np.ascontiguousarray(a[:, IN_COLS]) * np.float32(0.5)
    b_pre = np.ascontiguousarray(b[:, IN_COLS])
    a_bf = _to_bf16(a_pre)
    b_bf = _to_bf16(b_pre)

    o_dev = _run_device(nc, a_bf, b_bf, N_CORES)
    o_bf = np.asarray(o_dev)  # (n, 8) bf16, kernel position order

    o32 = (o_bf.view(np.uint16).astype(np.uint32) << np.uint32(16)).view(
        np.float32)
    out = np.empty((n, 8), np.float32)
    out[:, OUT_COLS] = o32
    return out


# revision 14
# speedup vs baseline: 1.9063x; 1.0090x over previous
"""Trainium2 Bass kernel for the Clifford (geometric) product on Cl(3,0).

out[n,k] = sum_{i,j} S[i,j,k] * a[n,i] * b[n,j]

Algorithm: Cl(3,0) ~= Mat2(C) (Pauli matrices). Per multivector:
  - basis elements indexed by GF(2)^3 masks; host permutes the 8 columns into
    a kernel-friendly order (and folds a global 1/2 scale into `a`)
  - 6 transform ops build the 2x2 complex matrix entries U/V (8 adds/mv each)
  - 8 product ops compute the 32 real products of the complex 2x2 matmul
  - a 3-level add/sub tree (structure signs folded into op choice) reduces to
    the 8 output components, written in a layout the host un-permutes
80 elementwise ops/mv total (vs 120 direct), every tree/transform op is
unit-stride inner (bf16 2x DVE mode); products are 1x (one broadcast operand)
and are split between the Vector and GpSimd engines by multivector range.

IO is bf16 (host rounds f32 inputs, upcasts the output); the rel-err budget
(2e-2) comfortably covers bf16 rounding. Host<->device runs through a cached
shard_map jit so repeat calls skip retracing, with the donated output buffer
created device-side (no zero upload).
"""

import os

os.environ.setdefault("BY_DEFAULT_DISABLE_SUBTILE_DEPS", "1")

import numpy as np
import ml_dtypes

import concourse.bass as bass
import concourse.bacc as bacc
import concourse.mybir as mybir
from concourse.tile import TileContext

# ---------------------------------------------------------------- geometry
N_TOTAL = 4194304
N_CORES = 8
NC = N_TOTAL // N_CORES        # 524288 multivectors per core
P = 128                        # partitions
E = 512                        # multivectors per partition per tile
EG = 0                         # mvs per tile on the GpSimd lane (0: DVE only —
                               # concurrent GpSimd SBUF traffic slows DVE more
                               # than the offload saves)
N_TILES = NC // (P * E)        # 8

F32 = mybir.dt.float32
BF16 = mybir.dt.bfloat16
ADD = mybir.AluOpType.add
SUB = mybir.AluOpType.subtract
MUL = mybir.AluOpType.mult

# host column gather/scatter (mask-space basis ordering; see module docstring)
IN_COLS = [0, 4, 1, 2, 3, 7, 5, 6]    # kernel input slot s <- ref column
OUT_COLS = [0, 4, 1, 2, 3, 7, 5, 6]   # kernel output pos p -> ref column

# ---- per-mv op tables (offsets/dims within one mv's 8-elem block) ----
# transforms: (alu, out_off, out_dims, in0_off, in0_dims, in1_off, in1_dims)
# U and V transforms share one structure (adds in slots 0-3, subs in 4-7)
TRANS = [
    (ADD, 0, [(1, 4)], 0, [(1, 4)], 4, [(1, 4)]),
    (SUB, 4, [(1, 4)], 2, [(-2, 2), (1, 2)], 6, [(-2, 2), (1, 2)]),
]
# products: U' (dup'd) slot pairs per term; V offsets (q//2)*2, t1/t3 use V2
USLOT = [(0, 2), (1, 3), (4, 6), (5, 7)]
# tree L2 part-patterns (sigma01 split)
L2_NEG = [(4, 2), (3, 2)]    # parts {0,3,4,7}
L2_POS = [(4, 2), (-1, 2)]   # parts (2,1,6,5) at offset +2 (4B-aligned base)


def _mkap(base, dims, offset):
    ap = base.copy()
    part = list(base.ap[0])
    ap.ap = mybir.VecI64Pair([part] + [[d, c] for (d, c) in dims])
    ap.offset = base.offset + offset
    return ap


# instruction name -> (label, elems); populated at build time for profiling
OP_NAMES = {}


def _reg(label, elems, inst):
    OP_NAMES[inst.ins.name] = (label, elems)
    return inst


def build_nc(nc_mv=NC, e=E, eg=EG):
    n_tiles = nc_mv // (P * e)
    assert n_tiles * P * e == nc_mv
    ed = e - eg

    nc = bacc.Bacc("TRN2", target_bir_lowering=False, debug=False)
    a_d = nc.dram_tensor("a", [nc_mv, 8], BF16, kind="ExternalInput")
    b_d = nc.dram_tensor("b", [nc_mv, 8], BF16, kind="ExternalInput")
    o_d = nc.dram_tensor("o", [nc_mv, 8], BF16, kind="ExternalOutput")

    a_v = a_d.ap().rearrange("(t p e) c -> t p (e c)", t=n_tiles, p=P)
    b_v = b_d.ap().rearrange("(t p e) c -> t p (e c)", t=n_tiles, p=P)
    o_v = o_d.ap().rearrange("(t p e) c -> t p (e c)", t=n_tiles, p=P)

    with TileContext(nc) as tc:
        with (
            tc.tile_pool(name="io", bufs=2) as io_pool,
            tc.tile_pool(name="uv", bufs=1) as uv_pool,
            tc.tile_pool(name="pr", bufs=2) as pr_pool,
        ):
            def emit_products(u2_t, v_t, v2_t, ranges, ptiles):
                for rname, eng, m0, cnt in ranges:
                    pts = ptiles[rname]
                    base = m0 * 8
                    for q in range(4):
                        vt = v_t if q % 2 == 0 else v2_t
                        voff = (q // 2) * 2
                        for half in range(2):
                            _reg(f"prod{q}h{half}{rname}", cnt * 4,
                                 eng.tensor_tensor(
                                out=_mkap(pts[q], [(8, cnt), (1, 4)], 4 * half),
                                in0=_mkap(u2_t, [(16, cnt), (0, 2), (1, 2)],
                                          m0 * 16 + 2 * USLOT[q][half]),
                                in1=_mkap(vt, [(8, cnt), (4, 2), (1, 2)],
                                          base + voff),
                                op=MUL))

            def emit_tree(tidx, ranges, ptiles, o_t):
                for rname, eng, m0, cnt in ranges:
                    pts = ptiles[rname]
                    c_t = pr_pool.tile([P, 8 * cnt], BF16, tag=f"c{rname}",
                                       name=f"c{rname}")
                    mv = [(8, cnt)]
                    V = nc.vector
                    _reg(f"L1a{rname}", cnt * 8,
                         V.tensor_tensor(out=_mkap(pts[0], mv + [(1, 8)], 0),
                                    in0=_mkap(pts[0], mv + [(1, 8)], 0),
                                    in1=_mkap(pts[2], mv + [(1, 8)], 0), op=ADD))
                    _reg(f"L1b{rname}", cnt * 8,
                         V.tensor_tensor(out=_mkap(pts[1], mv + [(1, 8)], 0),
                                    in0=_mkap(pts[1], mv + [(1, 8)], 0),
                                    in1=_mkap(pts[3], mv + [(1, 8)], 0), op=SUB))
                    _reg(f"L2a{rname}", cnt * 4,
                         V.tensor_tensor(out=_mkap(c_t, mv + L2_NEG, 0),
                                    in0=_mkap(pts[0], mv + L2_NEG, 0),
                                    in1=_mkap(pts[1], mv + L2_NEG, 0), op=SUB))
                    _reg(f"L2b{rname}", cnt * 4,
                         V.tensor_tensor(out=_mkap(c_t, mv + L2_POS, 2),
                                    in0=_mkap(pts[0], mv + L2_POS, 2),
                                    in1=_mkap(pts[1], mv + L2_POS, 2), op=ADD))
                    base = m0 * 8
                    _reg(f"finA{rname}", cnt * 4, V.tensor_tensor(
                        out=_mkap(o_t, mv + [(1, 4)], base),
                        in0=_mkap(c_t, mv + [(4, 2), (1, 2)], 0),
                        in1=_mkap(c_t, mv + [(-4, 2), (1, 2)], 6), op=ADD))
                    _reg(f"finB{rname}", cnt * 4, V.tensor_tensor(
                        out=_mkap(o_t, mv + [(1, 4)], base + 4),
                        in0=_mkap(c_t, mv + [(4, 2), (1, 2)], 0),
                        in1=_mkap(c_t, mv + [(-4, 2), (1, 2)], 6), op=SUB))
                nc.sync.dma_start(out=o_v[tidx], in_=o_t[:, :])

            Copy = mybir.ActivationFunctionType.Copy
            pending = None
            for t in range(n_tiles):
                a_t = io_pool.tile([P, 8 * e], BF16, tag="a")
                b_t = io_pool.tile([P, 8 * e], BF16, tag="b")
                o_t = io_pool.tile([P, 8 * e], BF16, tag="o")
                u_t = uv_pool.tile([P, 8 * e], BF16, tag="u")
                v_t = uv_pool.tile([P, 8 * e], BF16, tag="v")
                u2_t = uv_pool.tile([P, 16 * e], BF16, tag="u2")
                v2_t = uv_pool.tile([P, 8 * e], BF16, tag="v2")

                nc.sync.dma_start(out=a_t[:, :], in_=a_v[t])
                nc.scalar.dma_start(out=b_t[:, :], in_=b_v[t])

                # ---- transforms (DVE, full tile) ----
                for nm, src_t, dst_t in (("A", a_t, u_t), ("B", b_t, v_t)):
                    for ti, (alu, oo, od, i0o, i0d, i1o, i1d) in enumerate(TRANS):
                        _reg(f"trans{nm}{ti}", e * 4, nc.vector.tensor_tensor(
                            out=_mkap(dst_t, [(8, e)] + od, oo),
                            in0=_mkap(src_t, [(8, e)] + i0d, i0o),
                            in1=_mkap(src_t, [(8, e)] + i1d, i1o), op=alu))

                # ---- ACT copies (overlap with previous tile's tree) ----
                _reg("dupU", e * 16, nc.scalar.activation(
                    out=_mkap(u2_t, [(16, e), (2, 8), (1, 2)], 0),
                    in_=_mkap(u_t, [(8, e), (1, 8), (0, 2)], 0), func=Copy))
                _reg("swapV", e * 8, nc.scalar.activation(
                    out=_mkap(v2_t, [(8, e), (1, 8)], 0),
                    in_=_mkap(v_t, [(8, e), (2, 4), (-1, 2)], 1), func=Copy))

                # ---- previous tile's tree while ACT runs ----
                if pending is not None:
                    emit_tree(*pending)

                ranges = []
                ed = e - eg
                if ed > 0:
                    ranges.append(("d", nc.vector, 0, ed))
                if eg > 0:
                    ranges.append(("g", nc.gpsimd, ed, eg))
                ptiles = {}
                for rname, eng, m0, cnt in ranges:
                    ptiles[rname] = [
                        pr_pool.tile([P, 8 * cnt], BF16, tag=f"p{q}{rname}",
                                     name=f"p{q}{rname}")
                        for q in range(4)]
                emit_products(u2_t, v_t, v2_t, ranges, ptiles)
                pending = (t, ranges, ptiles, o_t)
            emit_tree(*pending)
    nc.compile()
    return nc


_NC_CACHE = {}


def _get_nc(nc_mv=NC, e=E, eg=EG):
    key = (nc_mv, e, eg)
    if key not in _NC_CACHE:
        _NC_CACHE[key] = build_nc(nc_mv, e, eg)
    return _NC_CACHE[key]


# ------------------------------------------------- cached PJRT execution
_EXEC_CACHE = {}


def _get_exec(nc, n_cores):
    """Cached equivalent of bass2jax.run_bass_via_pjrt: one traced+compiled
    shard_map jit per nc, with the donated output buffer made on device."""
    key = (id(nc), n_cores)
    if key in _EXEC_CACHE:
        return _EXEC_CACHE[key]

    import jax
    import jax.numpy as jnp
    from jax.experimental.shard_map import shard_map
    from jax.sharding import Mesh, PartitionSpec, NamedSharding
    from concourse import bass2jax

    bass2jax.install_neuronx_cc_hook()

    partition_name = (nc.partition_id_tensor.name
                      if nc.partition_id_tensor else None)
    in_names, out_names, out_avals = [], [], []
    for alloc in nc.m.functions[0].allocations:
        if not isinstance(alloc, mybir.MemoryLocationSet):
            continue
        name = alloc.memorylocations[0].name
        if alloc.kind == "ExternalInput":
            if name != partition_name:
                in_names.append(name)
        elif alloc.kind == "ExternalOutput":
            shape = tuple(alloc.tensor_shape)
            dtype = mybir.dt.np(alloc.dtype)
            out_names.append(name)
            out_avals.append(jax.core.ShapedArray(shape, dtype))
    n_params = len(in_names)
    n_outs = len(out_avals)
    all_names = in_names + out_names
    if partition_name is not None:
        all_names.append(partition_name)
    donate = tuple(range(n_params, n_params + n_outs))

    def _body(*args):
        operands = list(args)
        if partition_name is not None:
            operands.append(bass2jax.partition_id_tensor())
        outs = bass2jax._bass_exec_p.bind(
            *operands,
            out_avals=tuple(out_avals),
            in_names=tuple(all_names),
            out_names=tuple(out_names),
            lowering_input_output_aliases=(),
            sim_require_finite=True,
            sim_require_nnan=True,
            nc=nc,
        )
        return tuple(outs)

    devices = jax.devices()[:n_cores]
    mesh = Mesh(np.asarray(devices), ("core",))
    in_specs = (PartitionSpec("core"),) * (n_params + n_outs)
    out_specs = (PartitionSpec("core"),) * n_outs
    sharded = jax.jit(
        shard_map(_body, mesh=mesh, in_specs=in_specs, out_specs=out_specs,
                  check_rep=False),
        donate_argnums=donate, keep_unused=True)

    out_sharding = NamedSharding(mesh, PartitionSpec("core"))
    zero_fns = [
        jax.jit(
            (lambda av: (lambda: jnp.zeros((n_cores * av.shape[0],
                                            *av.shape[1:]), av.dtype)))(av),
            out_shardings=out_sharding)
        for av in out_avals
    ]
    entry = (sharded, zero_fns, in_names)
    _EXEC_CACHE[key] = entry
    return entry


def _run_device(nc, a_bf, b_bf, n_cores):
    by_name = {"a": a_bf, "b": b_bf}
    try:
        sharded, zero_fns, in_names = _get_exec(nc, n_cores)
        inputs = [by_name[nm] for nm in in_names]
        zeros = [zf() for zf in zero_fns]
        out_arrs = sharded(*inputs, *zeros)
        o = np.asarray(out_arrs[0])
    except Exception:
        # one retry for transient device faults (e.g. NRT exec-unit errors)
        _EXEC_CACHE.pop((id(nc), n_cores), None)
        sharded, zero_fns, in_names = _get_exec(nc, n_cores)
        inputs = [by_name[nm] for nm in in_names]
        zeros = [zf() for zf in zero_fns]
        out_arrs = sharded(*inputs, *zeros)
        o = np.asarray(out_arrs[0])
    return o


# ------------------------------------------------------------- host casts
_BF16 = ml_dtypes.bfloat16


def _to_bf16(x32):
    """f32 (contiguous) -> bf16 with round-to-nearest-even-ish rounding."""
    u = x32.view(np.uint32)
    r = ((u + np.uint32(0x7FFF) + ((u >> np.uint32(16)) & np.uint32(1)))
         >> np.uint32(16)).astype(np.uint16)
    return r.view(_BF16)


def kernel(a, b, M=None, **_):
    a = np.asarray(a, dtype=np.float32)
    b = np.asarray(b, dtype=np.float32)
    n = a.shape[0]
    assert n % N_CORES == 0
    nc_mv = n // N_CORES
    nc = _get_nc(nc_mv, E, EG)

    a_pre = np.ascontiguousarray(a[:, IN_COLS]) * np.float32(0.5)
    b_pre = np.ascontiguousarray(b[:, IN_COLS])
    a_bf = _to_bf16(a_pre)
    b_bf = _to_bf16(b_pre)

    o_bf = _run_device(nc, a_bf, b_bf, N_CORES)  # (n, 8) bf16, kernel order

    o32 = (o_bf.view(np.uint16).astype(np.uint32) << np.uint32(16)).view(
        np.float32)
    out = np.empty((n, 8), np.float32)
    out[:, OUT_COLS] = o32
    return out
